# revision 1
# baseline (speedup 1.0000x reference)
"""Bahdanau attention on 8 Trainium2 cores (Bass/Tile), data-parallel over B.

reference (per batch b, all shapes full):
    hp  = hidden[0] @ W_h.T + b_h                    # (B, H)
    ep  = einsum('tbh,gh->btg', enc, W_e) + b_e      # (B, T, H)
    en  = tanh(hp[:, None, :] + ep)                  # (B, T, H)
    sc  = en @ v                                     # (B, T)
    out = softmax(sc, -1)[:, None, :]                # (B, 1, T)

Sharding: B=32 split 4-per-core across 8 cores; W_h/W_e/b/v replicated.
Per-core kernel layout: tokens of one batch are processed in groups of 512;
enc tiles are PE-transposed to put H on partitions; ep accumulates over
8 h-chunks in PSUM as [g=128, tok=512] via fp32r matmuls (full PE rate,
~tf32 accuracy); ACT applies tanh with the per-partition bias
hp^T[:, b] + b_h + b_e; a [128x4] fp32r matmul against v4 (v in column b,
zeros elsewhere) reduces over g so batch b's scores land on PSUM partition
b; SBUF-to-SBUF DMA parks each batch's score row at partition 32*b, and
that batch's softmax + output DMA run incrementally as soon as its last
token group finishes, hiding the tail inside the main loop.
"""

import sys
from contextlib import ExitStack

import numpy as np

try:
    import concourse  # noqa: F401
except ImportError:  # pragma: no cover
    sys.path.insert(0, "/opt/trn_rl_repo")

import concourse.tile as tile
from concourse import bacc, mybir
from concourse.bass import ts
from concourse.bass_utils import run_bass_kernel_spmd
from concourse.masks import make_identity

H = 1024
T = 2048
B = 32
NCORES = 8
BC = B // NCORES          # batches per core
HC = H // 128             # h chunks
GC = H // 128             # g chunks
TOK = 512                 # tokens per group (one batch each)
SUB = TOK // 128          # 128-token subtiles per group
NGRP_PER_B = T // TOK
NGRP = BC * NGRP_PER_B

F32 = mybir.dt.float32
F32R = mybir.dt.float32r
AF = mybir.ActivationFunctionType
AX = mybir.AxisListType


def build_kernel_nc(reps=1):
    nc = bacc.Bacc(
        "TRN2",
        target_bir_lowering=False,
        debug=False,
        enable_asserts=False,
        num_devices=NCORES,
    )
    enc = nc.dram_tensor("enc", [T, BC, H], F32, kind="ExternalInput").ap()
    hid = nc.dram_tensor("hid", [BC, H], F32, kind="ExternalInput").ap()
    w_e = nc.dram_tensor("W_e", [H, H], F32, kind="ExternalInput").ap()
    w_h = nc.dram_tensor("W_h", [H, H], F32, kind="ExternalInput").ap()
    b_h = nc.dram_tensor("b_h", [H], F32, kind="ExternalInput").ap()
    b_e = nc.dram_tensor("b_e", [H], F32, kind="ExternalInput").ap()
    v = nc.dram_tensor("v", [H], F32, kind="ExternalInput").ap()
    out = nc.dram_tensor("out", [BC, T], F32, kind="ExternalOutput").ap()

    with tile.TileContext(nc) as tc:
        _kernel_body(tc, enc, hid, w_e, w_h, b_h, b_e, v, out, reps=reps)
    nc.compile()
    return nc


def _kernel_body(tc, enc, hid, w_e, w_h, b_h, b_e, v, out, reps=1):
    nc = tc.nc
    with ExitStack() as ctx:
        singles = ctx.enter_context(tc.tile_pool(name="singles", bufs=1))
        enc_pool = ctx.enter_context(tc.tile_pool(name="enc_nat", bufs=2 * SUB))
        encT_pool = ctx.enter_context(tc.tile_pool(name="encT", bufs=2))
        energy_pool = ctx.enter_context(tc.tile_pool(name="energy", bufs=3))
        scrow_pool = ctx.enter_context(tc.tile_pool(name="scrow", bufs=2))
        trps_pool = ctx.enter_context(
            tc.tile_pool(name="trps", bufs=3, space="PSUM")
        )
        ep_pool = ctx.enter_context(tc.tile_pool(name="epps", bufs=3, space="PSUM"))
        sc_pool = ctx.enter_context(tc.tile_pool(name="scps", bufs=2, space="PSUM"))

        identity = singles.tile([128, 128], F32)
        make_identity(nc, identity[:])

        # ---- persistent SBUF tensors -------------------------------------
        WeT = singles.tile([128, HC, H], F32R)     # WeT[h, hc, g] = W_e[g, 128*hc+h]
        WhT = singles.tile([128, HC, H], F32)
        hidT = singles.tile([128, HC, BC], F32)    # hidT[h, hc, b] = hid[b, 128*hc+h]
        bias_all = singles.tile([128, GC, BC], F32)  # hp^T + b_h + b_e
        v_sb = singles.tile([128, GC], F32)        # v[gc*128+p] at [p, gc]
        # v4[:, gc, b, :] is a [128, BC] stationary operand whose column b
        # holds the v chunk and the rest are zero -> batch b's scores land
        # on PSUM partition b (fp32r matmuls require dst partition 0).
        v4f = singles.tile([128, GC, BC, BC], F32)
        v4 = singles.tile([128, GC, BC, BC], F32R)
        bsum = singles.tile([128, GC], F32)        # (b_h + b_e) chunked
        # batch b's scores live on partition 32*b so per-batch softmax can
        # run as soon as that batch's groups finish (engine ops only accept
        # partition bases 0/32/64/96; DMA scatters the rows there)
        scores = singles.tile([128, T], F32)
        probs = singles.tile([128, T], F32)
        negmax = singles.tile([128, 1], F32)
        sums = singles.tile([128, 1], F32)
        rsum = singles.tile([128, 1], F32)

        # ---- stage 0: weights transpose + hp + biases --------------------
        bh_sb = singles.tile([128, GC], F32)
        be_sb = singles.tile([128, GC], F32)
        nc.sync.dma_start(out=bh_sb[:], in_=b_h.rearrange("(c p) -> p c", p=128))
        nc.sync.dma_start(out=be_sb[:], in_=b_e.rearrange("(c p) -> p c", p=128))
        nc.sync.dma_start(out=v_sb[:], in_=v.rearrange("(c p) -> p c", p=128))
        nc.vector.tensor_add(bsum[:], bh_sb[:], be_sb[:])
        nc.gpsimd.memset(v4f[:], 0.0)
        for b in range(BC):
            for gc in range(GC):
                nc.vector.tensor_copy(v4f[:, gc, b, b : b + 1], v_sb[:, gc : gc + 1])
        nc.vector.tensor_copy(v4[:], v4f[:])

        with tc.tile_pool(name="stage0", bufs=4) as wload:
            for w_src, w_dst in ((w_e, WeT), (w_h, WhT)):
                for gc in range(GC):
                    wn = wload.tile([128, H], F32, tag="wn")
                    nc.sync.dma_start(out=wn[:], in_=w_src[ts(gc, 128), :])
                    for hc in range(HC):
                        tp = trps_pool.tile([128, 128], F32, tag="tr")
                        nc.tensor.transpose(tp[:], wn[:, ts(hc, 128)], identity[:])
                        nc.vector.tensor_copy(w_dst[:, hc, ts(gc, 128)], tp[:])

            hid_nat = wload.tile([BC, H], F32, tag="hid")
            nc.sync.dma_start(out=hid_nat[:], in_=hid[:, :])
            for hc in range(HC):
                tph = trps_pool.tile([128, BC], F32, tag="tr")
                nc.tensor.transpose(
                    tph[:], hid_nat[:, ts(hc, 128)], identity[0:BC, 0:BC]
                )
                nc.vector.tensor_copy(hidT[:, hc, :], tph[:])

            # hp^T[g, b] accumulated over h chunks (fp32, tiny N)
            for gc in range(GC):
                hp_ps = trps_pool.tile([128, BC], F32, tag="tr")
                for hc in range(HC):
                    nc.tensor.matmul(
                        hp_ps[:],
                        WhT[:, hc, ts(gc, 128)],
                        hidT[:, hc, :],
                        start=(hc == 0),
                        stop=(hc == HC - 1),
                    )
                nc.vector.tensor_scalar(
                    out=bias_all[:, gc, :],
                    in0=hp_ps[:],
                    scalar1=bsum[:, gc : gc + 1],
                    scalar2=None,
                    op0=mybir.AluOpType.add,
                )

        # ---- main loop: 16 groups of 512 tokens --------------------------
        # Software-pipelined so the in-order PE queue never waits on ACT:
        #   iteration g emits: DMA(g+2), transposes(g+1), ep/sc chain(g)
        # with sc(gc-1) emitted after ep(gc) so tanh(gc-1) is long done.
        n_total = reps * NGRP

        def issue_load(grp):
            g = grp % NGRP
            b = g // NGRP_PER_B
            t0 = (g % NGRP_PER_B) * TOK
            en_nat = []
            for s in range(SUB):
                en = enc_pool.tile([128, H], F32, tag="en")
                nc.sync.dma_start(
                    out=en[:], in_=enc[t0 + s * 128 : t0 + (s + 1) * 128, b, :]
                )
                en_nat.append(en)
            return en_nat

        def issue_transposes(en_nat):
            encT = encT_pool.tile([128, HC, TOK], F32R)
            for hc in range(HC):
                tp = trps_pool.tile([128, TOK], F32, tag="tr")
                for s in range(SUB):
                    nc.tensor.transpose(
                        tp[:, ts(s, 128)], en_nat[s][:, ts(hc, 128)], identity[:]
                    )
                nc.vector.tensor_copy(encT[:, hc, :], tp[:])
            return encT

        loads = [issue_load(0), issue_load(1)]
        encT_cur = issue_transposes(loads[0])
        carry = None  # deferred final sc-mm of the previous group

        def softmax_b(b):
            r = slice(32 * b, 32 * b + 1)
            nc.vector.tensor_reduce(
                out=negmax[r], in_=scores[r, :], axis=AX.X,
                op=mybir.AluOpType.max, negate=True,
            )
            nc.scalar.activation(
                out=probs[r, :], in_=scores[r, :], func=AF.Exp,
                bias=negmax[r], scale=1.0, accum_out=sums[r],
            )
            nc.vector.reciprocal(out=rsum[r], in_=sums[r])
            nc.vector.tensor_scalar_mul(probs[r, :], probs[r, :], rsum[r])
            nc.sync.dma_start(out=out[b : b + 1, :], in_=probs[r, :])

        def flush_carry(c):
            c_sc_ps, c_gc, c_energy, c_b, c_t0 = c
            nc.tensor.matmul(
                c_sc_ps[:], v4[:, c_gc, c_b, :], c_energy[:],
                start=False, stop=True,
            )
            sc_sb = scrow_pool.tile([BC, TOK], F32)
            nc.vector.tensor_copy(sc_sb[:], c_sc_ps[:])
            nc.sync.dma_start(
                out=scores[32 * c_b : 32 * c_b + 1, c_t0 : c_t0 + TOK],
                in_=sc_sb[c_b : c_b + 1, :],
            )
            if c_t0 == T - TOK:
                softmax_b(c_b)

        for grp in range(n_total):
            g = grp % NGRP
            b = g // NGRP_PER_B
            t0 = (g % NGRP_PER_B) * TOK

            if grp + 2 < n_total:
                loads.append(issue_load(grp + 2))
            encT_next = None
            if grp + 1 < n_total:
                encT_next = issue_transposes(loads[grp + 1])
            if carry is not None:
                flush_carry(carry)
                carry = None

            sc_ps = sc_pool.tile([BC, TOK], F32)
            pending = None
            for gc in range(GC):
                ep_ps = ep_pool.tile([128, TOK], F32)
                for hc in range(HC):
                    nc.tensor.matmul(
                        ep_ps[:],
                        WeT[:, hc, ts(gc, 128)],
                        encT_cur[:, hc, :],
                        start=(hc == 0),
                        stop=(hc == HC - 1),
                    )
                if pending is not None:
                    pc, penergy = pending
                    nc.tensor.matmul(
                        sc_ps[:], v4[:, pc, b, :], penergy[:],
                        start=(pc == 0), stop=False,
                    )
                energy = energy_pool.tile([128, TOK], F32R)
                nc.scalar.activation(
                    out=energy[:],
                    in_=ep_ps[:],
                    func=AF.Tanh,
                    bias=bias_all[:, gc, b : b + 1],
                    scale=1.0,
                )
                pending = (gc, energy)
            pc, penergy = pending
            carry = (sc_ps, pc, penergy, b, t0)
            encT_cur = encT_next

        flush_carry(carry)


_NC_CACHE = None


def _get_nc():
    global _NC_CACHE
    if _NC_CACHE is None:
        _NC_CACHE = build_kernel_nc()
    return _NC_CACHE


def make_in_maps(hidden, encoder_outputs, W_h, b_h, W_e, b_e, v):
    hidden = np.asarray(hidden, dtype=np.float32)
    enc = np.asarray(encoder_outputs, dtype=np.float32)
    W_h = np.ascontiguousarray(np.asarray(W_h, dtype=np.float32))
    W_e = np.ascontiguousarray(np.asarray(W_e, dtype=np.float32))
    b_h = np.ascontiguousarray(np.asarray(b_h, dtype=np.float32))
    b_e = np.ascontiguousarray(np.asarray(b_e, dtype=np.float32))
    v = np.ascontiguousarray(np.asarray(v, dtype=np.float32))
    hid0 = hidden.reshape(B, H)
    in_maps = []
    for c in range(NCORES):
        in_maps.append(
            {
                "enc": np.ascontiguousarray(enc[:, c * BC : (c + 1) * BC, :]),
                "hid": np.ascontiguousarray(hid0[c * BC : (c + 1) * BC, :]),
                "W_e": W_e,
                "W_h": W_h,
                "b_h": b_h,
                "b_e": b_e,
                "v": v,
            }
        )
    return in_maps


def kernel(hidden, encoder_outputs, W_h, b_h, W_e, b_e, v):
    nc = _get_nc()
    in_maps = make_in_maps(hidden, encoder_outputs, W_h, b_h, W_e, b_e, v)
    res = run_bass_kernel_spmd(nc, in_maps, list(range(NCORES)))
    full = np.concatenate([res.results[c]["out"] for c in range(NCORES)], axis=0)
    return full[:, None, :].astype(np.float32)



# revision 3
# speedup vs baseline: 1.1924x; 1.1924x over previous
"""Bahdanau attention on 8 Trainium2 cores (Bass/Tile), data-parallel over B.

reference (per batch b, all shapes full):
    hp  = hidden[0] @ W_h.T + b_h                    # (B, H)
    ep  = einsum('tbh,gh->btg', enc, W_e) + b_e      # (B, T, H)
    en  = tanh(hp[:, None, :] + ep)                  # (B, T, H)
    sc  = en @ v                                     # (B, T)
    out = softmax(sc, -1)[:, None, :]                # (B, 1, T)

Sharding: B=32 split 4-per-core across 8 cores; weights replicated.

Per-core kernel: enc is shipped as bf16 (f32 hi-halves) and W_e/W_h are
shipped pre-transposed bf16, so the PE never runs a transpose. Each group
(one batch, 512 tokens) gets its enc tile delivered ALREADY transposed to
[h, tok] layout by the DMA XBAR transpose engine (dma_start_transpose,
16x128-tile hardware transpose, ~14ns/tile), fully overlapped with compute.
The PE stream is therefore nothing but back-to-back 512-column bf16
matmuls: 64 ep matmuls per group accumulating [g=128, tok=512] PSUM tiles
over 8 h-chunks, plus 8 score matmuls against v4 (v in column b so batch
b's scores land on PSUM partition b). ACT applies tanh with per-partition
bias hp^T[:, b] + b_h + b_e. The score matmul for energy chunk N is issued
behind the ep matmuls of chunk N+1 (also across group boundaries), so the
PE never waits on ACT. Scores are DMA-scattered to partition 32*b and each
batch's softmax + output DMA run as soon as its last group finishes.
"""

import sys
from contextlib import ExitStack

import numpy as np

try:
    import concourse  # noqa: F401
except ImportError:  # pragma: no cover
    sys.path.insert(0, "/opt/trn_rl_repo")

import concourse.tile as tile
from concourse import bacc, mybir
from concourse.bass import ts
from concourse.bass_utils import run_bass_kernel_spmd

H = 1024
T = 2048
B = 32
NCORES = 8
BC = B // NCORES          # batches per core
HC = H // 128             # h chunks
GC = H // 128             # g chunks
TOK = 512                 # tokens per group (one batch each)
NGRP_PER_B = T // TOK
NGRP = BC * NGRP_PER_B

F32 = mybir.dt.float32
BF16 = mybir.dt.bfloat16
AF = mybir.ActivationFunctionType
AX = mybir.AxisListType


def build_kernel_nc(reps=1):
    nc = bacc.Bacc(
        "TRN2",
        target_bir_lowering=False,
        debug=False,
        enable_asserts=False,
        num_devices=NCORES,
    )
    enc = nc.dram_tensor("enc", [T, BC, H], BF16, kind="ExternalInput").ap()
    hidT = nc.dram_tensor("hidT", [H, BC], BF16, kind="ExternalInput").ap()
    w_eT = nc.dram_tensor("W_eT", [H, H], BF16, kind="ExternalInput").ap()
    w_hT = nc.dram_tensor("W_hT", [H, H], BF16, kind="ExternalInput").ap()
    b_h = nc.dram_tensor("b_h", [H], F32, kind="ExternalInput").ap()
    b_e = nc.dram_tensor("b_e", [H], F32, kind="ExternalInput").ap()
    v = nc.dram_tensor("v", [H], F32, kind="ExternalInput").ap()
    out = nc.dram_tensor("out", [BC, T], F32, kind="ExternalOutput").ap()

    with tile.TileContext(nc) as tc:
        _kernel_body(tc, enc, hidT, w_eT, w_hT, b_h, b_e, v, out, reps=reps)
    nc.compile()
    return nc


def _kernel_body(tc, enc, hidT, w_eT, w_hT, b_h, b_e, v, out, reps=1):
    nc = tc.nc
    with ExitStack() as ctx:
        singles = ctx.enter_context(tc.tile_pool(name="singles", bufs=1))
        encT_pool = ctx.enter_context(tc.tile_pool(name="encT", bufs=3))
        energy_pool = ctx.enter_context(tc.tile_pool(name="energy", bufs=3))
        scrow_pool = ctx.enter_context(tc.tile_pool(name="scrow", bufs=2))
        ep_pool = ctx.enter_context(tc.tile_pool(name="epps", bufs=3, space="PSUM"))
        sc_pool = ctx.enter_context(tc.tile_pool(name="scps", bufs=2, space="PSUM"))

        # ---- persistent SBUF tensors -------------------------------------
        WeT = singles.tile([128, HC, H], BF16)     # WeT[h, hc, g] = W_e[g, 128*hc+h]
        bias_all = singles.tile([128, GC, BC], F32)  # hp^T + b_h + b_e
        v_sb = singles.tile([128, GC], F32)        # v[gc*128+p] at [p, gc]
        # v4[:, gc, b, :] is a [128, BC] stationary operand whose column b
        # holds the v chunk and the rest are zero -> batch b's scores land
        # on PSUM partition b.
        v4f = singles.tile([128, GC, BC, BC], F32)
        v4 = singles.tile([128, GC, BC, BC], BF16)
        bsum = singles.tile([128, GC], F32)        # (b_h + b_e) chunked
        # batch b's scores live on partition 32*b so per-batch softmax can
        # run as soon as that batch's groups finish (engine ops only accept
        # partition bases 0/32/64/96; DMA scatters the rows there)
        scores = singles.tile([128, T], F32)
        probs = singles.tile([128, T], F32)
        negmax = singles.tile([128, 1], F32)
        sums = singles.tile([128, 1], F32)
        rsum = singles.tile([128, 1], F32)

        # ---- stage 0: weight loads + hp + biases -------------------------
        bh_sb = singles.tile([128, GC], F32)
        be_sb = singles.tile([128, GC], F32)
        nc.sync.dma_start(out=bh_sb[:], in_=b_h.rearrange("(c p) -> p c", p=128))
        nc.sync.dma_start(out=be_sb[:], in_=b_e.rearrange("(c p) -> p c", p=128))
        nc.sync.dma_start(out=v_sb[:], in_=v.rearrange("(c p) -> p c", p=128))
        nc.vector.tensor_add(bsum[:], bh_sb[:], be_sb[:])
        nc.gpsimd.memset(v4f[:], 0.0)
        for b in range(BC):
            for gc in range(GC):
                nc.vector.tensor_copy(v4f[:, gc, b, b : b + 1], v_sb[:, gc : gc + 1])
        nc.vector.tensor_copy(v4[:], v4f[:])

        nc.sync.dma_start(
            out=WeT[:], in_=w_eT.rearrange("(hc p) g -> p hc g", p=128)
        )

        with ExitStack() as s0:
            wload = s0.enter_context(tc.tile_pool(name="stage0", bufs=1))
            hp_pool = s0.enter_context(
                tc.tile_pool(name="hpps", bufs=2, space="PSUM")
            )
            WhT = wload.tile([128, HC, H], BF16)
            hidT_sb = wload.tile([128, HC, BC], BF16)
            nc.sync.dma_start(
                out=WhT[:], in_=w_hT.rearrange("(hc p) g -> p hc g", p=128)
            )
            nc.sync.dma_start(
                out=hidT_sb[:], in_=hidT.rearrange("(hc p) b -> p hc b", p=128)
            )
            # hp^T[g, b] accumulated over h chunks
            for gc in range(GC):
                hp_ps = hp_pool.tile([128, BC], F32)
                for hc in range(HC):
                    nc.tensor.matmul(
                        hp_ps[:],
                        WhT[:, hc, ts(gc, 128)],
                        hidT_sb[:, hc, :],
                        start=(hc == 0),
                        stop=(hc == HC - 1),
                    )
                nc.vector.tensor_scalar(
                    out=bias_all[:, gc, :],
                    in0=hp_ps[:],
                    scalar1=bsum[:, gc : gc + 1],
                    scalar2=None,
                    op0=mybir.AluOpType.add,
                )

        # ---- main loop: 16 groups of 512 tokens --------------------------
        # PE stream is pure matmuls; the sc matmul of energy chunk N issues
        # behind the ep matmuls of chunk N+1 (incl. across group bounds).
        n_total = reps * NGRP

        def issue_load(grp):
            g = grp % NGRP
            b = g // NGRP_PER_B
            t0 = (g % NGRP_PER_B) * TOK
            # One XBAR-transpose instruction moves the whole [TOK, H] slab
            # into [h, hc, tok] layout: out[p, hc, t] = in[t, 128*hc + p].
            encT = encT_pool.tile([128, HC, TOK], BF16)
            nc.sync.dma_start_transpose(
                out=encT[:], in_=enc[t0 : t0 + TOK, b, :]
            )
            return encT

        def softmax_b(b):
            r = slice(32 * b, 32 * b + 1)
            nc.vector.tensor_reduce(
                out=negmax[r], in_=scores[r, :], axis=AX.X,
                op=mybir.AluOpType.max, negate=True,
            )
            nc.scalar.activation(
                out=probs[r, :], in_=scores[r, :], func=AF.Exp,
                bias=negmax[r], scale=1.0, accum_out=sums[r],
            )
            nc.vector.reciprocal(out=rsum[r], in_=sums[r])
            nc.vector.tensor_scalar_mul(probs[r, :], probs[r, :], rsum[r])
            nc.sync.dma_start(out=out[b : b + 1, :], in_=probs[r, :])

        def finish_group(c_sc_ps, c_b, c_t0):
            sc_sb = scrow_pool.tile([BC, TOK], F32)
            nc.vector.tensor_copy(sc_sb[:], c_sc_ps[:])
            nc.sync.dma_start(
                out=scores[32 * c_b : 32 * c_b + 1, c_t0 : c_t0 + TOK],
                in_=sc_sb[c_b : c_b + 1, :],
            )
            if c_t0 == T - TOK:
                softmax_b(c_b)

        loads = [issue_load(0), issue_load(1)]
        # pending = (sc_ps, gc, energy, b, t0) score-matmul not yet issued
        pending = None

        for grp in range(n_total):
            g = grp % NGRP
            b = g // NGRP_PER_B
            t0 = (g % NGRP_PER_B) * TOK

            if grp + 2 < n_total:
                loads.append(issue_load(grp + 2))
            encT_cur = loads[grp]

            sc_ps = sc_pool.tile([BC, TOK], F32)
            for gc in range(GC):
                ep_ps = ep_pool.tile([128, TOK], F32)
                for hc in range(HC):
                    nc.tensor.matmul(
                        ep_ps[:],
                        WeT[:, hc, ts(gc, 128)],
                        encT_cur[:, hc, :],
                        start=(hc == 0),
                        stop=(hc == HC - 1),
                    )
                if pending is not None:
                    p_sc_ps, p_gc, p_energy, p_b, p_t0 = pending
                    nc.tensor.matmul(
                        p_sc_ps[:], v4[:, p_gc, p_b, :], p_energy[:],
                        start=(p_gc == 0), stop=(p_gc == GC - 1),
                    )
                    if p_gc == GC - 1:
                        finish_group(p_sc_ps, p_b, p_t0)
                energy = energy_pool.tile([128, TOK], BF16)
                nc.scalar.activation(
                    out=energy[:],
                    in_=ep_ps[:],
                    func=AF.Tanh,
                    bias=bias_all[:, gc, b : b + 1],
                    scale=1.0,
                )
                pending = (sc_ps, gc, energy, b, t0)

        # drain the last pending score matmul
        p_sc_ps, p_gc, p_energy, p_b, p_t0 = pending
        nc.tensor.matmul(
            p_sc_ps[:], v4[:, p_gc, p_b, :], p_energy[:],
            start=(p_gc == 0), stop=True,
        )
        finish_group(p_sc_ps, p_b, p_t0)


_NC_CACHE = None


def _get_nc():
    global _NC_CACHE
    if _NC_CACHE is None:
        _NC_CACHE = build_kernel_nc()
    return _NC_CACHE


def _to_bf16(a):
    """Round-to-nearest-even f32 -> bf16, returned as ml_dtypes.bfloat16."""
    import ml_dtypes

    return np.asarray(a, dtype=np.float32).astype(ml_dtypes.bfloat16)


def _truncate_bf16(a):
    """Truncate f32 -> bf16 by taking the high 16 bits (cheap view+slice)."""
    import ml_dtypes

    a = np.ascontiguousarray(np.asarray(a, dtype=np.float32))
    hi = a.view(np.uint16)[..., 1::2]
    return np.ascontiguousarray(hi).view(ml_dtypes.bfloat16)


def make_in_maps(hidden, encoder_outputs, W_h, b_h, W_e, b_e, v):
    enc = _truncate_bf16(encoder_outputs)            # (T, B, H) bf16
    hidT = np.ascontiguousarray(
        _to_bf16(np.asarray(hidden, np.float32).reshape(B, H)).T
    )                                                # (H, B) bf16
    W_eT = np.ascontiguousarray(_to_bf16(W_e).T)     # (H, H) bf16, [h, g]
    W_hT = np.ascontiguousarray(_to_bf16(W_h).T)
    b_h = np.ascontiguousarray(np.asarray(b_h, dtype=np.float32))
    b_e = np.ascontiguousarray(np.asarray(b_e, dtype=np.float32))
    v = np.ascontiguousarray(np.asarray(v, dtype=np.float32))
    in_maps = []
    for c in range(NCORES):
        in_maps.append(
            {
                "enc": np.ascontiguousarray(enc[:, c * BC : (c + 1) * BC, :]),
                "hidT": np.ascontiguousarray(hidT[:, c * BC : (c + 1) * BC]),
                "W_eT": W_eT,
                "W_hT": W_hT,
                "b_h": b_h,
                "b_e": b_e,
                "v": v,
            }
        )
    return in_maps


def kernel(hidden, encoder_outputs, W_h, b_h, W_e, b_e, v):
    nc = _get_nc()
    in_maps = make_in_maps(hidden, encoder_outputs, W_h, b_h, W_e, b_e, v)
    res = run_bass_kernel_spmd(nc, in_maps, list(range(NCORES)))
    full = np.concatenate([res.results[c]["out"] for c in range(NCORES)], axis=0)
    return full[:, None, :].astype(np.float32)


# revision 13
# speedup vs baseline: 1.4668x; 1.2302x over previous
"""Bahdanau attention on 8 Trainium2 cores (Bass/Tile), data-parallel over B.

reference (per batch b, all shapes full):
    hp  = hidden[0] @ W_h.T + b_h                    # (B, H)
    ep  = einsum('tbh,gh->btg', enc, W_e) + b_e      # (B, T, H)
    en  = tanh(hp[:, None, :] + ep)                  # (B, T, H)
    sc  = en @ v                                     # (B, T)
    out = softmax(sc, -1)[:, None, :]                # (B, 1, T)

Sharding: B=32 split 4-per-core across 8 cores; weights replicated.

Per-core kernel (epT layout): enc is shipped bf16 (f32 hi-halves) and
weights pre-transposed bf16. The DMA XBAR transpose engine delivers each
group's enc slab already transposed to [h, hc, tok] (one instruction per
512-token group, fully overlapped). The PE stream is then NOTHING but the
ep matmuls: stationary = encT 128-token chunk (8 LDW per token tile),
moving = WeT rows, output epT[t=128, g=1024] in PSUM — 218us of pure
back-to-back 512-col bf16 matmuls per pass, nothing else on the PE's
critical path. DVE adds the per-batch bias row (hp[b] + b_h + b_e,
replicated across partitions at stage 0 via ones-matmuls), ACT applies
tanh, and DVE's fused tensor_tensor_reduce multiplies by v and reduces
over the free axis g, accumulating scores into scT[b] = [tok%128, 16].
Each batch's softmax (tiny PE transposes to cross partitions + ACT exp)
is deferred two groups so the PE never waits, and the output row is
DMA'd from a [16, 128] tile.
"""

import sys
from contextlib import ExitStack

import numpy as np

try:
    import concourse  # noqa: F401
except ImportError:  # pragma: no cover
    sys.path.insert(0, "/opt/trn_rl_repo")

import concourse.tile as tile
from concourse import bacc, mybir
from concourse.bass import ts
from concourse.bass_utils import run_bass_kernel_spmd
from concourse.masks import make_identity

H = 1024
T = 2048
B = 32
NCORES = 8
BC = B // NCORES          # batches per core
HC = H // 128             # h chunks
TOK = 512                 # tokens per group (one batch each)
SUB = TOK // 128          # 128-token stationary tiles per group
NGRP_PER_B = T // TOK
NGRP = BC * NGRP_PER_B
NTILE_B = T // 128        # token tiles per batch (16)

F32 = mybir.dt.float32
BF16 = mybir.dt.bfloat16
AF = mybir.ActivationFunctionType
AX = mybir.AxisListType
ALU = mybir.AluOpType


def build_kernel_nc(reps=1):
    nc = bacc.Bacc(
        "TRN2",
        target_bir_lowering=False,
        debug=False,
        enable_asserts=False,
        num_devices=NCORES,
    )
    enc = nc.dram_tensor("enc", [T, BC, H], BF16, kind="ExternalInput").ap()
    hidT = nc.dram_tensor("hidT", [H, BC], BF16, kind="ExternalInput").ap()
    w_eT = nc.dram_tensor("W_eT", [H, H], BF16, kind="ExternalInput").ap()
    w_hT = nc.dram_tensor("W_hT", [H, H], BF16, kind="ExternalInput").ap()
    b_h = nc.dram_tensor("b_h", [H], F32, kind="ExternalInput").ap()
    b_e = nc.dram_tensor("b_e", [H], F32, kind="ExternalInput").ap()
    v = nc.dram_tensor("v", [H], F32, kind="ExternalInput").ap()
    out = nc.dram_tensor("out", [BC, T], F32, kind="ExternalOutput").ap()

    with tile.TileContext(nc) as tc:
        _kernel_body(tc, enc, hidT, w_eT, w_hT, b_h, b_e, v, out, reps=reps)
    nc.compile()
    return nc


def _kernel_body(tc, enc, hidT, w_eT, w_hT, b_h, b_e, v, out, reps=1):
    nc = tc.nc
    with ExitStack() as ctx:
        singles = ctx.enter_context(tc.tile_pool(name="singles", bufs=1))
        encT_pool = ctx.enter_context(tc.tile_pool(name="encT", bufs=3))
        pre_pool = ctx.enter_context(tc.tile_pool(name="pre", bufs=3))
        tanh_pool = ctx.enter_context(tc.tile_pool(name="tanh", bufs=3))
        vout_pool = ctx.enter_context(tc.tile_pool(name="vout", bufs=2))
        fin_pool = ctx.enter_context(tc.tile_pool(name="fin", bufs=2))

        # ---- persistent SBUF tensors -------------------------------------
        WeT = singles.tile([128, HC, H], BF16)     # WeT[h, hc, g] = W_e[g, 128*hc+h]
        v_rep = singles.tile([128, H], BF16)       # v broadcast across partitions
        bias_rep = singles.tile([128, BC, H], BF16)  # (hp[b] + b_h + b_e) bcast
        # scT[p, b, i]: score of token 128*i + p of batch b
        scT = singles.tile([128, BC, NTILE_B], F32)
        identity = singles.tile([128, 128], F32)
        ones_row = singles.tile([1, 128], F32)     # K=1 stationary for bcasts
        ones_col = singles.tile([128, 1], F32)     # moving ones for part-sums
        negV_bc = singles.tile([NTILE_B, 1], F32)  # -sum|v|: safe exp shift

        make_identity(nc, identity[:])
        nc.vector.memset(ones_row[:], 1.0)
        nc.vector.memset(ones_col[:], 1.0)

        nc.sync.dma_start(
            out=WeT[:], in_=w_eT.rearrange("(hc p) g -> p hc g", p=128)
        )

        # ---- stage 0: v_rep, bias rows -----------------------------------
        with ExitStack() as s0:
            wload = s0.enter_context(tc.tile_pool(name="stage0", bufs=1))
            s0_ps = s0.enter_context(tc.tile_pool(name="s0ps", bufs=2, space="PSUM"))

            v_row = wload.tile([1, H], F32)
            bh_row = wload.tile([1, H], F32)
            be_row = wload.tile([1, H], F32)
            bsum_row = wload.tile([1, H], F32)
            nc.sync.dma_start(out=v_row[:], in_=v.rearrange("(o h) -> o h", o=1))
            nc.sync.dma_start(out=bh_row[:], in_=b_h.rearrange("(o h) -> o h", o=1))
            nc.sync.dma_start(out=be_row[:], in_=b_e.rearrange("(o h) -> o h", o=1))
            nc.vector.tensor_add(bsum_row[:], bh_row[:], be_row[:])

            # negV = -sum|v|: |score| <= sum|v|, so exp(score - V) never
            # overflows; a constant shift replaces the per-batch max.
            vabs_row = wload.tile([1, H], F32)
            vsum = wload.tile([1, 1], F32)
            nc.scalar.activation(
                out=vabs_row[:], in_=v_row[:], func=AF.Abs,
                scale=1.0, accum_out=vsum[:],
            )
            negv_ps = s0_ps.tile([NTILE_B, 1], F32, tag="neg")
            nc.tensor.matmul(
                negv_ps[:], ones_row[:, 0:NTILE_B], vsum[:],
                start=True, stop=True,
            )
            nc.vector.tensor_scalar_mul(negV_bc[:], negv_ps[:], -1.0)

            # v_rep = ones(128,1) @ v_row
            for half in range(2):
                vb_ps = s0_ps.tile([128, 512], F32, tag="b512")
                nc.tensor.matmul(
                    vb_ps[:], ones_row[:], v_row[:, ts(half, 512)],
                    start=True, stop=True,
                )
                nc.vector.tensor_copy(v_rep[:, ts(half, 512)], vb_ps[:])

            # hp rows: hp[b, g] = sum_h hid[b,h] WhT[h,g]  (+ bsum via K=1 mm)
            WhT = wload.tile([128, HC, H], BF16)
            hidT_sb = wload.tile([128, HC, BC], BF16)
            nc.sync.dma_start(
                out=WhT[:], in_=w_hT.rearrange("(hc p) g -> p hc g", p=128)
            )
            nc.sync.dma_start(
                out=hidT_sb[:], in_=hidT.rearrange("(hc p) b -> p hc b", p=128)
            )
            bias_rows = wload.tile([BC, H], F32)
            for half in range(2):
                hp_ps = s0_ps.tile([BC, 512], F32, tag="hp")
                for hc in range(HC):
                    nc.tensor.matmul(
                        hp_ps[:],
                        hidT_sb[:, hc, :],
                        WhT[:, hc, ts(half, 512)],
                        start=(hc == 0),
                        stop=False,
                    )
                nc.tensor.matmul(
                    hp_ps[:],
                    ones_row[:, 0:BC],
                    bsum_row[:, ts(half, 512)],
                    start=False,
                    stop=True,
                )
                nc.vector.tensor_copy(bias_rows[:, ts(half, 512)], hp_ps[:])

            # move each bias row onto partition 0 (DMA has no base-partition
            # restriction), then broadcast to all 128 partitions via
            # ones-matmuls (engine operands must sit at base 0)
            bias_sp = wload.tile([1, BC, H], F32)
            for b in range(BC):
                nc.sync.dma_start(
                    out=bias_sp[0:1, b, :],
                    in_=bias_rows[b : b + 1, :],
                )
            for b in range(BC):
                for half in range(2):
                    bb_ps = s0_ps.tile([128, 512], F32, tag="b512")
                    nc.tensor.matmul(
                        bb_ps[:],
                        ones_row[:],
                        bias_sp[0:1, b, ts(half, 512)],
                        start=True, stop=True,
                    )
                    nc.vector.tensor_copy(
                        bias_rep[:, b, ts(half, 512)], bb_ps[:]
                    )

        ep_pool = ctx.enter_context(tc.tile_pool(name="epps", bufs=3, space="PSUM"))
        fin_ps_pool = ctx.enter_context(
            tc.tile_pool(name="finps", bufs=1, space="PSUM")
        )

        # ---- per-batch softmax finalization ------------------------------
        def finalize_batch(b):
            # scT[:, b, :] is [128, 16]: token 128*i + p. One PE transpose
            # crosses partitions; exp uses the constant -sum|v| shift; the
            # partition-sum of the per-partition exp sums is a tiny matmul.
            sc_tp = fin_ps_pool.tile([NTILE_B, 128], F32, tag="sctp")
            nc.tensor.transpose(sc_tp[:], scT[:, b, :], identity[:])
            sc_nat = fin_pool.tile([NTILE_B, 128], F32, tag="nat")
            nc.vector.tensor_copy(sc_nat[:], sc_tp[:])

            probs_nat = fin_pool.tile([NTILE_B, 128], F32, tag="probs")
            sums16 = fin_pool.tile([NTILE_B, 1], F32, tag="sums")
            nc.scalar.activation(
                out=probs_nat[:], in_=sc_nat[:], func=AF.Exp,
                bias=negV_bc[:], scale=1.0, accum_out=sums16[:],
            )
            # small_ps[0:1, 0:1] = total; small_ps[:, 1:2] = bcast reciprocal
            small_ps = fin_ps_pool.tile([NTILE_B, 2], F32, tag="small")
            nc.tensor.matmul(
                small_ps[0:1, 0:1], sums16[:], ones_col[0:NTILE_B, :],
                start=True, stop=True,
            )
            rtot = fin_pool.tile([1, 1], F32, tag="rtot")
            nc.vector.reciprocal(out=rtot[:], in_=small_ps[0:1, 0:1])
            nc.tensor.matmul(
                small_ps[:, 1:2], ones_row[:, 0:NTILE_B], rtot[:],
                start=True, stop=True,
            )
            rbc = fin_pool.tile([NTILE_B, 1], F32, tag="rbc")
            nc.vector.tensor_copy(rbc[:], small_ps[:, 1:2])
            nc.vector.tensor_scalar_mul(probs_nat[:], probs_nat[:], rbc[:])
            nc.sync.dma_start(
                out=out[b : b + 1, :].rearrange("o (q f) -> (o q) f", q=NTILE_B),
                in_=probs_nat[:],
            )

        # ---- main loop: 16 groups of 512 tokens per rep ------------------
        n_total = reps * NGRP

        def issue_load(grp):
            g = grp % NGRP
            b = g // NGRP_PER_B
            t0 = (g % NGRP_PER_B) * TOK
            # One XBAR-transpose instruction moves the whole [TOK, H] slab
            # into [h, hc, tok] layout: out[p, hc, t] = in[t, 128*hc + p].
            encT = encT_pool.tile([128, HC, TOK], BF16)
            nc.sync.dma_start_transpose(
                out=encT[:], in_=enc[t0 : t0 + TOK, b, :]
            )
            return encT

        loads = [issue_load(0), issue_load(1)]
        finals_due = {}  # grp index -> batch to finalize at that group start

        for grp in range(n_total):
            g = grp % NGRP
            b = g // NGRP_PER_B
            t0 = (g % NGRP_PER_B) * TOK

            if grp + 2 < n_total:
                loads.append(issue_load(grp + 2))
            encT_cur = loads[grp]

            fb = finals_due.pop(grp, None)
            if fb is not None:
                finalize_batch(fb)

            for s in range(SUB):
                tidx = (g % NGRP_PER_B) * SUB + s   # token tile in batch
                ep_ps = ep_pool.tile([128, H], F32)
                for hc in range(HC):
                    for half in range(2):
                        nc.tensor.matmul(
                            ep_ps[:, ts(half, 512)],
                            encT_cur[:, hc, ts(s, 128)],
                            WeT[:, hc, ts(half, 512)],
                            start=(hc == 0),
                            stop=(hc == HC - 1),
                        )
                pre = pre_pool.tile([128, H], BF16)
                nc.vector.scalar_tensor_tensor(
                    out=pre[:],
                    in0=ep_ps[:],
                    scalar=1.0,
                    in1=bias_rep[:, b, :],
                    op0=ALU.mult,
                    op1=ALU.add,
                )
                en = tanh_pool.tile([128, H], BF16)
                nc.scalar.activation(
                    out=en[:], in_=pre[:], func=AF.Tanh, scale=1.0,
                )
                vo = vout_pool.tile([128, H], BF16)
                nc.vector.scalar_tensor_tensor(
                    out=vo[:],
                    in0=en[:],
                    scalar=1.0,
                    in1=v_rep[:],
                    op0=ALU.mult,
                    op1=ALU.mult,
                    accum_out=scT[:, b, tidx : tidx + 1],
                )

            if g % NGRP_PER_B == NGRP_PER_B - 1:
                # batch b's scores complete (in flight); finalize 2 groups on
                finals_due[grp + 2] = b

        # drain remaining finals
        for grp in sorted(finals_due):
            finalize_batch(finals_due[grp])


_NC_CACHE = None


def _get_nc():
    global _NC_CACHE
    if _NC_CACHE is None:
        _NC_CACHE = build_kernel_nc()
    return _NC_CACHE


def _to_bf16(a):
    """Round-to-nearest-even f32 -> bf16, returned as ml_dtypes.bfloat16."""
    import ml_dtypes

    return np.asarray(a, dtype=np.float32).astype(ml_dtypes.bfloat16)


def _truncate_bf16(a):
    """Truncate f32 -> bf16 by taking the high 16 bits (cheap view+slice)."""
    import ml_dtypes

    a = np.ascontiguousarray(np.asarray(a, dtype=np.float32))
    hi = a.view(np.uint16)[..., 1::2]
    return np.ascontiguousarray(hi).view(ml_dtypes.bfloat16)


def make_in_maps(hidden, encoder_outputs, W_h, b_h, W_e, b_e, v):
    enc = _truncate_bf16(encoder_outputs)            # (T, B, H) bf16
    hidT = np.ascontiguousarray(
        _to_bf16(np.asarray(hidden, np.float32).reshape(B, H)).T
    )                                                # (H, B) bf16
    W_eT = np.ascontiguousarray(_to_bf16(W_e).T)     # (H, H) bf16, [h, g]
    W_hT = np.ascontiguousarray(_to_bf16(W_h).T)
    b_h = np.ascontiguousarray(np.asarray(b_h, dtype=np.float32))
    b_e = np.ascontiguousarray(np.asarray(b_e, dtype=np.float32))
    v = np.ascontiguousarray(np.asarray(v, dtype=np.float32))
    in_maps = []
    for c in range(NCORES):
        in_maps.append(
            {
                "enc": np.ascontiguousarray(enc[:, c * BC : (c + 1) * BC, :]),
                "hidT": np.ascontiguousarray(hidT[:, c * BC : (c + 1) * BC]),
                "W_eT": W_eT,
                "W_hT": W_hT,
                "b_h": b_h,
                "b_e": b_e,
                "v": v,
            }
        )
    return in_maps


def kernel(hidden, encoder_outputs, W_h, b_h, W_e, b_e, v):
    nc = _get_nc()
    in_maps = make_in_maps(hidden, encoder_outputs, W_h, b_h, W_e, b_e, v)
    res = run_bass_kernel_spmd(nc, in_maps, list(range(NCORES)))
    full = np.concatenate([res.results[c]["out"] for c in range(NCORES)], axis=0)
    return full[:, None, :].astype(np.float32)
